# revision 40
# baseline (speedup 1.0000x reference)
"""Causal self-attention block (B=32, T=512, C=768, H=12) on 8 Trainium2 cores.

Strategy: data-parallel over batch (4 batches per core). All matmuls in bf16
with fp32 PSUM accumulation. The dataflow is arranged so no on-chip
transposes are needed:

  host:   xT[b] = x[b].T  (bf16, [C, T])
  qkT  [2C, T] = W_qk.T @ x.T      (lhsT = natural W_qk slices, rhs = xT)
  v    [T, C]  = x @ W_v           (lhsT = xT slices,           rhs = W_v)
  sT_h [Tk, Tq] = k_h q_h.T        (lhsT = kT_h slice,  rhs = qT_h slice, K=D)
  pT_h = exp(sT/sqrt(D)) * causal  (scalar engine; no max-sub needed: |s/8|<~2)
  o_h  [D+1, Tq] = [v_h | 1].T @ pT_h   (row D = softmax denominator l)
  oT_h = o_h[:D] * (1/l)           (K=1 matmul broadcasts 1/l over partitions)
  y    [T, C]  = o @ W_proj        (lhsT = oT slices, rhs = natural W_proj)

Causality is exploited at tile granularity: for k-tile i only q >= 128*i is
computed; the diagonal 128x128 chunk is masked with a 0/1 bf16 mask.
"""
import sys

sys.path.insert(0, "/opt/trn_rl_repo")

import numpy as np
import ml_dtypes

import concourse.bass as bass
import concourse.tile as tile
import concourse.mybir as mybir
from concourse import bacc, library_config
from concourse.bass_utils import run_bass_kernel_spmd

F32 = mybir.dt.float32
F32R = mybir.dt.float32r
BF16 = mybir.dt.bfloat16
AF = mybir.ActivationFunctionType
BF16NP = ml_dtypes.bfloat16

B, T, C = 32, 512, 768
H = 12
D = C // H  # 64
NCORES = 8
NB = B // NCORES  # batches per core
KT = C // 128  # 6 contraction tiles
MQK = (2 * C) // 128  # 12 output tiles for q|k features
TT = T // 128  # 4 token tiles
SCALE = 1.0 / np.sqrt(D)


DEFAULT_CFG = dict(
    xt=2, qkt=2, va=2, ot=2, pt=6, small=6, ysb=3, mm=2, st=2, o=2, y=2,
    ou=2, norm_pool=0, pairs=0,
)


def build_bass(cfg=None):
    cfg = {**DEFAULT_CFG, **(cfg or {})}
    nc = bacc.Bacc()

    xT_d = nc.dram_tensor("xT", [NB, C, T], BF16, kind="ExternalInput")
    wqk_d = nc.dram_tensor("wqk", [C, 2 * C], BF16, kind="ExternalInput")
    wv_d = nc.dram_tensor("wv", [C, C], BF16, kind="ExternalInput")
    wp_d = nc.dram_tensor("wp", [C, C], BF16, kind="ExternalInput")
    bqk_d = nc.dram_tensor("bqk", [128, MQK], F32, kind="ExternalInput")
    bv_d = nc.dram_tensor("bv", [128, C], F32, kind="ExternalInput")
    bp_d = nc.dram_tensor("bp", [128, C], F32, kind="ExternalInput")
    mask_d = nc.dram_tensor("mask", [128, 128], BF16, kind="ExternalInput")
    y_d = nc.dram_tensor("y", [NB, T, C], F32, kind="ExternalOutput")

    with tile.TileContext(nc) as tc:
        with (
            tc.tile_pool(name="consts", bufs=1) as consts,
            tc.tile_pool(name="xt", bufs=cfg["xt"]) as xt_pool,
            tc.tile_pool(name="qkt", bufs=cfg["qkt"]) as qkt_pool,
            tc.tile_pool(name="va", bufs=cfg["va"]) as va_pool,
            tc.tile_pool(name="ot", bufs=cfg["ot"]) as ot_pool,
            tc.tile_pool(name="pt", bufs=cfg["pt"]) as pt_pool,
            tc.tile_pool(name="small", bufs=cfg["small"]) as small_pool,
            tc.tile_pool(name="ysb", bufs=cfg["ysb"]) as y_pool,
            tc.tile_pool(name="psmm", bufs=cfg["mm"], space="PSUM") as ps_mm,
            tc.tile_pool(name="psst", bufs=cfg["st"], space="PSUM") as ps_st,
            tc.tile_pool(name="pso", bufs=cfg["o"], space="PSUM") as ps_o,
            tc.tile_pool(name="psy", bufs=max(cfg["y"], 1), space="PSUM") as ps_y,
        ):
            # ---- constants (issue order = need order: XT[0], Wqk, then rest) ----
            XT0 = xt_pool.tile([128, KT, T], BF16, tag="xt")
            xt0_r = xT_d[0].rearrange("(k p) t -> p k t", p=128)
            Wqk = consts.tile([128, KT, 2 * C], BF16)
            wqk_r = wqk_d.rearrange("(k p) n -> p k n", p=128)
            for k in range(KT):
                nc.sync.dma_start(XT0[:, k, :], xt0_r[:, k, :])
                nc.sync.dma_start(Wqk[:, k, :], wqk_r[:, k, :])
            Bqk = consts.tile([128, MQK], F32)
            nc.sync.dma_start(Bqk, bqk_d[:])
            Wv = consts.tile([128, KT, C], BF16)
            nc.sync.dma_start(Wv, wv_d.rearrange("(k p) n -> p k n", p=128))
            Mask = consts.tile([128, 128], BF16)
            nc.sync.dma_start(Mask, mask_d[:])
            Bv = consts.tile([128, C], F32)
            nc.sync.dma_start(Bv, bv_d[:])
            Wp = consts.tile([128, KT, C], BF16)
            nc.sync.dma_start(Wp, wp_d.rearrange("(k p) n -> p k n", p=128))
            Bp = consts.tile([128, C], F32)
            nc.sync.dma_start(Bp, bp_d[:])
            nc.gpsimd.load_library(library_config.attn)

            for b in range(NB):
                # ---- load xT for this batch ----
                if b == 0:
                    XT = XT0
                else:
                    XT = xt_pool.tile([128, KT, T], BF16, tag="xt")
                    nc.sync.dma_start(XT, xT_d[b].rearrange("(k p) t -> p k t", p=128))

                # ---- GEMM1: qkT [2C, T], feature-major ----
                # emit m-tiles in (q-tile, k-tile) pairs so head h unblocks
                # after 2 m-tiles instead of after the whole q half
                QKT = qkt_pool.tile([128, MQK, T], BF16)
                m_order = [m for qt_ in range(MQK // 2) for m in (qt_, MQK // 2 + qt_)]
                for m in m_order:
                    qk_ps = ps_mm.tile([128, T], F32, tag="mm")
                    for k in range(KT):
                        nc.tensor.matmul(
                            qk_ps,
                            Wqk[:, k, 128 * m : 128 * (m + 1)],
                            XT[:, k, :],
                            start=(k == 0),
                            stop=(k == KT - 1),
                        )
                    nc.scalar.activation(
                        QKT[:, m, :], qk_ps, AF.Identity, bias=Bqk[:, m : m + 1]
                    )

                # ---- GEMM2: v_aug [T, H, D+1], token-major with ones column ----
                VA = va_pool.tile([128, TT, H, D + 1], BF16)
                nc.vector.memset(VA[:, :, :, D : D + 1], 1.0)
                for t in range(TT):
                    for n0, nw in ((0, 512), (512, 256)):
                        v_ps = ps_mm.tile([128, T], F32, tag="mm")
                        for k in range(KT):
                            nc.tensor.matmul(
                                v_ps[:, :nw],
                                XT[:, k, 128 * t : 128 * (t + 1)],
                                Wv[:, k, n0 : n0 + nw],
                                start=(k == 0),
                                stop=(k == KT - 1),
                            )
                        nc.vector.tensor_tensor(
                            VA[:, t, n0 // D : (n0 + nw) // D, 0:D],
                            v_ps[:, :nw].rearrange("p (h d) -> p h d", d=D),
                            Bv[:, n0 : n0 + nw].rearrange("p (h d) -> p h d", d=D),
                            mybir.AluOpType.add,
                        )

                # ---- attention per head ----
                OT = ot_pool.tile([128, KT, T], BF16)

                def head_slices(h):
                    qt = h // 2
                    qr = D * (h % 2)
                    return (
                        QKT[qr : qr + D, qt, :],
                        QKT[qr : qr + D, MQK // 2 + qt, :],
                    )

                def st_exp_av(h, i, o_ps):
                    qT_h, kT_h = head_slices(h)
                    n = T - 128 * i
                    st_ps = ps_st.tile([128, T], F32, tag="st")
                    nc.tensor.matmul(
                        st_ps[:, :n],
                        kT_h[:, 128 * i : 128 * (i + 1)],
                        qT_h[:, 128 * i : T],
                        start=True,
                        stop=True,
                    )
                    PT = pt_pool.tile([128, T], BF16)
                    nc.scalar.activation(PT[:, :n], st_ps[:, :n], AF.Exp, scale=SCALE)
                    nc.vector.tensor_tensor(
                        PT[:, 0:128], PT[:, 0:128], Mask, mybir.AluOpType.mult
                    )
                    nc.tensor.matmul(
                        o_ps[0 : D + 1, 128 * i : T],
                        VA[:, i, h, :],
                        PT[:, :n],
                        start=(i == 0),
                        stop=(i == TT - 1),
                    )

                def normalize(h, o_ps):
                    # normalize: oT_h = o[:D] * (1/l), l = o row D
                    qt = h // 2
                    qr = D * (h % 2)
                    rinv = small_pool.tile([1, T], F32, tag="rinv")
                    nc.vector.reciprocal(rinv, o_ps[D : D + 1, :])
                    rb = small_pool.tile([D, T], F32, tag="rb_sb")
                    nc.gpsimd.partition_broadcast(rb, rinv[:])
                    if cfg["ou"] == 0:
                        nc.vector.tensor_tensor(
                            OT[qr : qr + D, qt, :],
                            o_ps[0:D, :],
                            rb,
                            mybir.AluOpType.mult,
                        )
                    else:
                        oU = small_pool.tile([D, T], F32, tag="ou_sb")
                        if cfg["ou"] == 1:
                            nc.scalar.copy(oU, o_ps[0:D, :])
                        else:
                            nc.vector.tensor_copy(oU, o_ps[0:D, :])
                        eng = nc.gpsimd if cfg["norm_pool"] else nc.vector
                        eng.tensor_tensor(
                            OT[qr : qr + D, qt, :],
                            oU,
                            rb,
                            mybir.AluOpType.mult,
                        )

                if cfg["pairs"]:
                    # paired emission: the two heads of a QKT tile alternate, so
                    # their K=64 ST matmuls sit adjacently at row groups 0/64
                    # (concurrent on HW via tile_position row packing)
                    for pair in range(H // 2):
                        hA, hB = 2 * pair, 2 * pair + 1
                        oA = ps_o.tile([128, T], F32, tag="o")
                        oB = ps_o.tile([128, T], F32, tag="o")
                        for i in range(TT):
                            st_exp_av(hA, i, oA)
                            st_exp_av(hB, i, oB)
                        normalize(hA, oA)
                        normalize(hB, oB)
                else:
                    for h in range(H):
                        o_ps = ps_o.tile([128, T], F32, tag="o")
                        for i in range(TT):
                            st_exp_av(h, i, o_ps)
                        normalize(h, o_ps)

                # ---- GEMM4: y = o @ W_proj + b ----
                for t in range(TT):
                    y_sb = y_pool.tile([128, C], F32)
                    for n0, nw in ((0, 512), (512, 256)):
                        if cfg["y"] == 0:
                            y_ps = ps_mm.tile([128, T], F32, tag="mm")
                        else:
                            y_ps = ps_y.tile([128, T], F32, tag="y")
                        for k in range(KT):
                            nc.tensor.matmul(
                                y_ps[:, :nw],
                                OT[:, k, 128 * t : 128 * (t + 1)],
                                Wp[:, k, n0 : n0 + nw],
                                start=(k == 0),
                                stop=(k == KT - 1),
                            )
                        nc.vector.tensor_tensor(
                            y_sb[:, n0 : n0 + nw],
                            y_ps[:, :nw],
                            Bp[:, n0 : n0 + nw],
                            mybir.AluOpType.add,
                        )
                    nc.sync.dma_start(y_d[b, 128 * t : 128 * (t + 1), :], y_sb)

    return nc


_NC_CACHE = None


def _get_nc():
    global _NC_CACHE
    if _NC_CACHE is None:
        nc = build_bass()
        nc.finalize()
        _NC_CACHE = nc
    return _NC_CACHE


def make_in_maps(x, w_qkv, b_qkv, b_proj, w_proj):
    wqk = np.ascontiguousarray(w_qkv[:, : 2 * C]).astype(BF16NP)
    wv = np.ascontiguousarray(w_qkv[:, 2 * C :]).astype(BF16NP)
    wp = np.asarray(w_proj).astype(BF16NP)
    bqk = np.ascontiguousarray(
        np.asarray(b_qkv[: 2 * C], np.float32).reshape(MQK, 128).T
    )
    bv = np.broadcast_to(np.asarray(b_qkv[2 * C :], np.float32), (128, C)).copy()
    bp = np.broadcast_to(np.asarray(b_proj, np.float32), (128, C)).copy()
    kk, qq = np.meshgrid(np.arange(128), np.arange(128), indexing="ij")
    mask = (kk <= qq).astype(BF16NP)

    in_maps = []
    for c in range(NCORES):
        xc = np.asarray(x[c * NB : (c + 1) * NB], np.float32)
        xT = np.ascontiguousarray(xc.transpose(0, 2, 1)).astype(BF16NP)
        in_maps.append(
            {
                "xT": xT,
                "wqk": wqk,
                "wv": wv,
                "wp": wp,
                "bqk": bqk,
                "bv": bv,
                "bp": bp,
                "mask": mask,
            }
        )
    return in_maps


def kernel(x, w_qkv, b_qkv, w_proj, b_proj, _trace=False, _tmpdir=None):
    x = np.asarray(x)
    in_maps = make_in_maps(x, w_qkv, b_qkv, b_proj, w_proj)
    nc = _get_nc()
    res = run_bass_kernel_spmd(
        nc, in_maps, list(range(NCORES)), trace=_trace, tmpdir=_tmpdir
    )
    out = np.concatenate([np.asarray(r["y"], np.float32) for r in res.results], axis=0)
    if _trace:
        kernel.last_exec_time_ns = res.exec_time_ns
        kernel.last_results = res
    return out.reshape(B, T, C)


if __name__ == "__main__":
    rng = np.random.default_rng(0)
    x = rng.standard_normal((B, T, C), dtype=np.float32)
    w_qkv = (rng.standard_normal((C, 3 * C), dtype=np.float32) * 0.02).astype(np.float32)
    b_qkv = np.zeros((3 * C,), np.float32)
    w_proj = (rng.standard_normal((C, C), dtype=np.float32) * 0.02).astype(np.float32)
    b_proj = np.zeros((C,), np.float32)
    y = kernel(x, w_qkv=w_qkv, b_qkv=b_qkv, w_proj=w_proj, b_proj=b_proj)
    print(y.shape, y.dtype)
